# revision 22
# baseline (speedup 1.0000x reference)
"""AFT-Full forward on 8 Trainium2 NeuronCores.

Sharding: core c -> (batch b = c//2, output-time-half h = c%2).
Each core computes out[b, h*1024:(h+1)*1024, :] with no cross-core
communication. Host-side work is only layout prep (transpose / roll /
tile / dtype cast) and the final gather.

Per-core math (T=2048, D=1024, H=256, Th=1024 = this core's t-half):
  Q^T   = Wq^T @ x_b^T[:, t-half]    [H, Th]    (fp32r matmul)
  sQ    = sigmoid(Q^T + bq)
  K|V   = x_b @ [Wk|Wv]              [T, 512]   (fp32r matmul, f32 psum)
  eK    = exp(K + bk), eKV = eK*(V + bv)        stored [s, h] in SBUF
  den^T = sum_s eK[s,h] * ew^T[s,t]             (fp32r matmul)
  num^T = same with eKV                         (fp32r matmul)
  Yt^T  = sQ * num^T / den^T
  out^T = Wp^T @ Yt^T + bp           [D, Th]    (fp32r matmul)

The t-axis of x^T and the s-axis of wbias^T are rolled by -h*1024 per
core so "this core's t-half" is always columns 0:1024 of the rolled
frame; sums over s are order-invariant so the roll is harmless.

All DRAM parameters are host-pre-tiled to [128, ...] partition-major
layout so every DMA is a plain 2D copy with large contiguous runs
(HWDGE descriptor generation on the sync sequencer is the head-latency
bottleneck otherwise).
"""

import sys

for _p in ("/opt/trn_rl_repo",):
    if _p not in sys.path:
        sys.path.insert(0, _p)

import numpy as np
import ml_dtypes

import concourse.bacc as bacc
import concourse.tile as tile
from concourse import mybir
from concourse.bass_utils import run_bass_kernel_spmd

BF16 = ml_dtypes.bfloat16

B, T, DIM, HID = 4, 2048, 1024, 256
TH = T // 2          # per-core t-half
N_CORES = 8
P = 128              # partitions
ND = DIM // P        # 8 d-tiles
NT = T // P          # 16 t(/s)-tiles
NH = HID // P        # 2 h-tiles
NM = DIM // P        # 8 output dim-tiles
CH = 512             # matmul moving free-dim chunk
NC_CH = TH // CH     # 2 chunks per t-half
WBG = 4              # wbias s-tiles per batched DMA
OG = 1               # m-tiles per staged output DMA
F32 = mybir.dt.float32
F32R = mybir.dt.float32r
DBF = mybir.dt.bfloat16
AF = mybir.ActivationFunctionType


def _tile_rows(a, np_dtype):
    """[G*128, N] -> [128, G*N] partition-major, contiguous."""
    g = a.shape[0] // P
    return np.ascontiguousarray(
        a.reshape(g, P, a.shape[1]).transpose(1, 0, 2).reshape(P, -1)
    ).astype(np_dtype)


def _build():
    nc = bacc.Bacc(None, target_bir_lowering=False)

    xt_ext = nc.declare_dram_parameter("xt", [P, NT * ND * P], F32R,
                                       isOutput=False)
    wq_ext = nc.declare_dram_parameter("wq", [P, ND * HID], F32R, isOutput=False)
    wkv_ext = nc.declare_dram_parameter("wkv", [P, ND * 2 * HID], F32R,
                                        isOutput=False)
    wp_ext = nc.declare_dram_parameter("wp", [P, NH * DIM], F32R, isOutput=False)
    wbt_ext = nc.declare_dram_parameter("wbt", [P, NT * TH], DBF, isOutput=False)
    bias_ext = nc.declare_dram_parameter("bias", [P, 522], F32, isOutput=False)
    out_ext = nc.declare_dram_parameter("outT", [DIM, TH], F32, isOutput=True)

    with tile.TileContext(nc) as tc:
        with (
            tc.tile_pool(name="persist", bufs=1) as pp,
            tc.tile_pool(name="stream", bufs=3) as sp,
            tc.tile_pool(name="evac", bufs=3) as ep,
        ):
            # ---- resident SBUF tensors (same pre-tiled layouts) ----
            xt = pp.tile([P, NT, ND, P], F32R, tag="xt")
            wq = pp.tile([P, ND, HID], F32R, tag="wq")
            wkv = pp.tile([P, ND, 2 * HID], F32R, tag="wkv")
            wp = pp.tile([P, NH, DIM], F32R, tag="wp")
            bias = pp.tile([P, 522], F32, tag="bias")
            ekvk = pp.tile([P, NT, 2 * HID], F32R, tag="ekvk")  # eK | eKV
            sq = pp.tile([P, NH, TH], F32, tag="sq")
            yt = pp.tile([P, NH, TH], F32R, tag="yt")
            bq2 = bias[:, 0:NH]
            bkv = bias[:, NH:NH + 2 * HID]
            bp8 = bias[:, NH + 2 * HID:522]

            # ---- DMAs, ordered by first use (HWDGE FIFO on sync) ----
            wkv_r = wkv_ext.rearrange("p (n h) -> p n h", n=ND)
            nc.sync.dma_start(wkv[:, 0:ND // 2, :], wkv_r[:, 0:ND // 2, :])
            BB = ND * P  # elements per xt block
            nc.sync.dma_start(xt[:, 0, :, :], xt_ext[:, 0:BB])
            nc.sync.dma_start(xt[:, 1, :, :], xt_ext[:, BB:2 * BB])
            nc.sync.dma_start(wkv[:, ND // 2:ND, :], wkv_r[:, ND // 2:ND, :])
            nc.sync.dma_start(bias[:, :], bias_ext[:, :])
            for i in range(2, NT):
                nc.sync.dma_start(xt[:, i, :, :], xt_ext[:, i * BB:(i + 1) * BB])
            nc.sync.dma_start(wq[:, :, :],
                              wq_ext.rearrange("p (n h) -> p n h", n=ND))
            # wbias^T batches AFTER x on the same sync FIFO: issuing them
            # on a parallel queue makes the SDMA engines round-robin them
            # against the latency-critical x stream at packet granularity
            wbts = []
            for g in range(NT // WBG):
                wbt = sp.tile([P, WBG, TH], DBF, tag="wbt", bufs=2)
                nc.sync.dma_start(
                    wbt[:, :, :],
                    wbt_ext.rearrange("p (g t) -> p g t", g=NT)[
                        :, g * WBG:(g + 1) * WBG, :],
                )
                wbts.append(wbt)
            nc.sync.dma_start(wp[:, :, :],
                              wp_ext.rearrange("p (u m) -> p u m", u=NH))

            ws = pp.tile([P, CH], DBF, tag="ws")
            nc.vector.memset(ws[:, :].bitcast(F32), 0.0)

            with tc.tile_pool(name="ps1", bufs=1, space="PSUM") as ps1:
                # PE warmup: dummy matmuls with no DMA deps keep the HAM
                # activity window busy while the first x blocks stream in,
                # so the first real matmuls run at 2.4 GHz instead of 1.2
                pw = ps1.tile([P, CH], F32, tag="warm")
                for w in range(12):
                    nc.tensor.matmul(pw[:, :], ws[:, 0:P], ws[:, :],
                                     start=True, stop=True)

                # ---- phase 1a: K|V, eK, eKV (block i arrives -> tile i) ----
                for i in range(NT):
                    pkv = ps1.tile([P, 2 * HID], F32, tag="pkv", bufs=3)
                    for n in range(ND):
                        nc.tensor.matmul(
                            pkv[:, :],
                            xt[:, i, n, :],
                            wkv[:, n, :],
                            start=(n == 0),
                            stop=(n == ND - 1),
                        )
                    kvb = sp.tile([P, 2 * HID], F32, tag="kvb", bufs=2)
                    nc.vector.tensor_add(kvb[:, :], pkv[:, :], bkv[:, :])
                    nc.scalar.activation(
                        ekvk[:, i, 0:HID], kvb[:, 0:HID], AF.Exp
                    )
                    nc.vector.tensor_mul(
                        ekvk[:, i, HID:2 * HID], ekvk[:, i, 0:HID],
                        kvb[:, HID:2 * HID],
                    )

                # ---- phase 1b: Q^T = Wq^T @ x^T[:, 0:TH] ----
                pqts = [
                    ps1.tile([P, TH], F32, tag=f"pqt{u}", name=f"pqt{u}")
                    for u in range(NH)
                ]
                for u in range(NH):
                    for n in range(ND):
                        for c in range(NC_CH):
                            nc.tensor.matmul(
                                pqts[u][:, c * CH:(c + 1) * CH],
                                wq[:, n, u * P:(u + 1) * P],
                                xt[:, 4 * c:4 * (c + 1), n, :],
                                start=(n == 0),
                                stop=(n == ND - 1),
                            )
                    # sigmoid(Q+bq) = 1/(1+exp(-Q-bq)): keeps ACT on the
                    # Exp table (a Sigmoid table swap costs ~1.5us each way)
                    sge = sp.tile([P, TH], F32, tag="sge", bufs=1,
                                  name=f"sge{u}")
                    nc.scalar.activation(
                        sge[:, :], pqts[u][:, :], AF.Exp,
                        bias=bq2[:, u:u + 1], scale=-1.0,
                    )
                    nc.vector.tensor_scalar_add(sge[:, :], sge[:, :], 1.0)
                    nc.vector.reciprocal_approx_fast(sq[:, u, :], sge[:, :])

                # pre-exp the first two ew tiles so phase 2 can start
                # immediately after Q^T (ACT is FIFO)
                ews = {}
                for st in range(2):
                    ew = sp.tile([P, TH], F32R, tag="ew", bufs=2,
                                 name=f"ew{st}")
                    nc.scalar.activation(
                        ew[:, :], wbts[st // WBG][:, st % WBG, :], AF.Exp
                    )
                    ews[st] = ew

            # ---- phase 2: den^T (acc0/1) and num^T (acc2/3) ----
            with tc.tile_pool(name="ps2", bufs=1, space="PSUM") as ps2:
                # 8 one-bank accumulator tiles: acc[a][c] for quadrant a,
                # chunk c. Finer granularity gives phase 3 an 8-slot ring.
                accs = [
                    [
                        ps2.tile([P, CH], F32, tag=f"acc{a}c{c}",
                                 name=f"acc{a}c{c}")
                        for c in range(NC_CH)
                    ]
                    for a in range(4)
                ]
                for st in range(NT):
                    if st in ews:
                        ew = ews[st]
                    else:
                        ew = sp.tile([P, TH], F32R, tag="ew", bufs=2,
                                     name=f"ew{st}")
                        nc.scalar.activation(
                            ew[:, :], wbts[st // WBG][:, st % WBG, :], AF.Exp
                        )
                    for a in range(4):
                        u = a % 2
                        base = (a // 2) * HID  # 0 -> eK(den), HID -> eKV(num)
                        lh = ekvk[:, st, base + u * P: base + (u + 1) * P]
                        for c in range(NC_CH):
                            nc.tensor.matmul(
                                accs[a][c][:, :],
                                lh,
                                ew[:, c * CH:(c + 1) * CH],
                                start=(st == 0),
                                stop=(st == NT - 1),
                            )

                # ---- epilogue: Yt^T = sQ * num^T / den^T (chunked) ----
                # recips (DVE, from den psum) run alongside ACT copying num
                # to SBUF so the multiplies hit DVE's 2x fp32 SBUF mode
                nsbs = []
                for u in range(NH):
                    nsb = sp.tile([P, TH], F32, tag="nsb", bufs=2,
                                  name=f"nsb{u}")
                    for c in range(NC_CH):
                        nc.scalar.copy(nsb[:, c * CH:(c + 1) * CH],
                                       accs[2 + u][c][:, :])
                    nsbs.append(nsb)
                first = True
                for c in range(NC_CH):
                    recs = []
                    for u in range(NH):
                        r = sp.tile([P, CH], F32, tag="rec", bufs=2,
                                    name=f"rec{u}{c}")
                        nc.vector.reciprocal_approx_fast(
                            r[:, :], accs[u][c][:, :]
                        )
                        recs.append(r)
                    if first:
                        # dummy matmuls keep HAM warm across the epilogue's
                        # PE-idle window (den c0 slots just freed)
                        for a in range(2):
                            pwd = ps2.tile([P, CH], F32, tag=f"acc{a}c0",
                                           name=f"warm2{a}")
                            nc.tensor.matmul(pwd[:, :], ws[:, 0:P], ws[:, :],
                                             start=True, stop=True)
                        first = False
                    for u in range(NH):
                        cs = slice(c * CH, (c + 1) * CH)
                        tmp = sp.tile([P, CH], F32, tag="tmp", bufs=2)
                        nc.vector.tensor_mul(tmp[:, :], nsbs[u][:, cs],
                                             recs[u][:, :])
                        nc.vector.tensor_mul(yt[:, u, cs], tmp[:, :],
                                             sq[:, u, cs])

                # ---- phase 3: out^T = Wp^T @ Yt^T + bp ----
                # m-outer: each Wp stationary tile is loaded once and used
                # for both 512-chunks; psum slots recycle the 4 acc tags
                out_r = out_ext.rearrange("(m p) t -> p m t", p=P)
                ptags = [f"acc{a}c{c}" for a in range(4) for c in range(NC_CH)]
                for mg in range(NM // OG):
                    ob = ep.tile([P, OG, TH], F32, tag="ob", bufs=3,
                                 name=f"ob{mg}")
                    for k in range(OG):
                        m = mg * OG + k
                        pos = [
                            ps2.tile([P, CH], F32,
                                     tag=ptags[(2 * m + c) % 8],
                                     name=f"po{c}{m}")
                            for c in range(NC_CH)
                        ]
                        for u in range(NH):
                            for c in range(NC_CH):
                                nc.tensor.matmul(
                                    pos[c][:, :],
                                    wp[:, u, m * P:(m + 1) * P],
                                    yt[:, u, c * CH:(c + 1) * CH],
                                    start=(u == 0),
                                    stop=(u == NH - 1),
                                )
                        for c in range(NC_CH):
                            if (m + c) % 2 == 0:
                                nc.scalar.add(ob[:, k, c * CH:(c + 1) * CH],
                                              pos[c][:, :], bp8[:, m:m + 1])
                            else:
                                nc.vector.tensor_scalar_add(
                                    ob[:, k, c * CH:(c + 1) * CH],
                                    pos[c][:, :], bp8[:, m:m + 1]
                                )
                    nc.sync.dma_start(
                        out_r[:, mg * OG:(mg + 1) * OG, :],
                        ob[:, :, :],
                    )

    nc.finalize()
    return nc


_NC = None


def _get_nc():
    global _NC
    if _NC is None:
        _NC = _build()
    return _NC


def _make_in_maps(x, Wq, bq, Wk, bk, Wv, bv, Wp, bp, wbias):
    wq = _tile_rows(np.asarray(Wq, np.float32), np.float32)
    wkv = _tile_rows(
        np.concatenate([Wk, Wv], axis=1).astype(np.float32), np.float32
    )
    wp = _tile_rows(np.asarray(Wp, np.float32), np.float32)
    bias = np.zeros((P, 522), np.float32)
    bias[:, 0:NH] = -np.asarray(bq, np.float32).reshape(NH, P).T
    bias[:, NH:NH + 2 * HID] = np.concatenate([bk, bv]).astype(np.float32)
    bias[:, NH + 2 * HID:] = np.asarray(bp, np.float32).reshape(NM, P).T
    wb = np.asarray(wbias, np.float32)[:T, :T]

    in_maps = []
    for c in range(N_CORES):
        b, half = divmod(c, 2)
        toff = half * TH
        xr = np.roll(np.asarray(x[b], np.float32).T, -toff, axis=1)
        # [P, t-block i, n, col] so one 512KB DMA unlocks one K/V tile
        xt = np.ascontiguousarray(
            xr.reshape(ND, P, NT, P).transpose(1, 2, 0, 3).reshape(P, -1)
        )
        # ew^T[s_rolled, j] = wbias[toff + j, (s_rolled + toff) % T]
        wbt = np.ascontiguousarray(
            np.roll(wb[toff:toff + TH, :], -toff, axis=1).T
        )
        wbt = _tile_rows(wbt, BF16)
        in_maps.append({
            "xt": xt, "wq": wq, "wkv": wkv, "wp": wp, "wbt": wbt,
            "bias": bias,
        })
    return in_maps


def run_on_hw(in_maps, trace=False):
    nc = _get_nc()
    return run_bass_kernel_spmd(
        nc, in_maps, core_ids=list(range(N_CORES)), trace=trace
    )


def kernel(**inputs) -> np.ndarray:
    in_maps = _make_in_maps(**inputs)
    res = run_on_hw(in_maps, trace=False)
    out = np.empty((B, T, DIM), dtype=np.float32)
    for c in range(N_CORES):
        b, half = divmod(c, 2)
        toff = half * TH
        out[b, toff:toff + TH, :] = res.results[c]["outT"].T.astype(np.float32)
    return out


# revision 24
# speedup vs baseline: 1.0479x; 1.0479x over previous
"""AFT-Full forward on 8 Trainium2 NeuronCores.

Sharding: core c -> (batch b = c//2, output-time-half h = c%2).
Each core computes out[b, h*1024:(h+1)*1024, :] with no cross-core
communication. Host-side work is only layout prep (transpose / roll /
tile / dtype cast) and the final gather.

Per-core math (T=2048, D=1024, H=256, Th=1024 = this core's t-half):
  Q^T   = Wq^T @ x_b^T[:, t-half]    [H, Th]    (fp32r matmul)
  sQ    = sigmoid(Q^T + bq)
  K|V   = x_b @ [Wk|Wv]              [T, 512]   (fp32r matmul, f32 psum)
  eK    = exp(K + bk), eKV = eK*(V + bv)        stored [s, h] in SBUF
  den^T = sum_s eK[s,h] * ew^T[s,t]             (fp32r matmul)
  num^T = same with eKV                         (fp32r matmul)
  Yt^T  = sQ * num^T / den^T
  out^T = Wp^T @ Yt^T + bp           [D, Th]    (fp32r matmul)

The t-axis of x^T and the s-axis of wbias^T are rolled by -h*1024 per
core so "this core's t-half" is always columns 0:1024 of the rolled
frame; sums over s are order-invariant so the roll is harmless.

All DRAM parameters are host-pre-tiled to [128, ...] partition-major
layout so every DMA is a plain 2D copy with large contiguous runs
(HWDGE descriptor generation on the sync sequencer is the head-latency
bottleneck otherwise).
"""

import sys

for _p in ("/opt/trn_rl_repo",):
    if _p not in sys.path:
        sys.path.insert(0, _p)

import numpy as np
import ml_dtypes

import concourse.bacc as bacc
import concourse.tile as tile
from concourse import mybir
from concourse.bass_utils import run_bass_kernel_spmd

BF16 = ml_dtypes.bfloat16

B, T, DIM, HID = 4, 2048, 1024, 256
TH = T // 2          # per-core t-half
N_CORES = 8
P = 128              # partitions
ND = DIM // P        # 8 d-tiles
NT = T // P          # 16 t(/s)-tiles
NH = HID // P        # 2 h-tiles
NM = DIM // P        # 8 output dim-tiles
CH = 512             # matmul moving free-dim chunk
NC_CH = TH // CH     # 2 chunks per t-half
WBG = 4              # wbias s-tiles per batched DMA
OG = 1               # m-tiles per staged output DMA
F32 = mybir.dt.float32
F32R = mybir.dt.float32r
DBF = mybir.dt.bfloat16
AF = mybir.ActivationFunctionType


def _tile_rows(a, np_dtype):
    """[G*128, N] -> [128, G*N] partition-major, contiguous."""
    g = a.shape[0] // P
    return np.ascontiguousarray(
        a.reshape(g, P, a.shape[1]).transpose(1, 0, 2).reshape(P, -1)
    ).astype(np_dtype)


def _build():
    nc = bacc.Bacc(None, target_bir_lowering=False)

    xt_ext = nc.declare_dram_parameter("xt", [P, NT * ND * P], F32R,
                                       isOutput=False)
    wq_ext = nc.declare_dram_parameter("wq", [P, ND * HID], F32R, isOutput=False)
    wkv_ext = nc.declare_dram_parameter("wkv", [P, ND * 2 * HID], F32R,
                                        isOutput=False)
    wp_ext = nc.declare_dram_parameter("wp", [P, NH * DIM], F32R, isOutput=False)
    wbt_ext = nc.declare_dram_parameter("wbt", [P, NT * TH], DBF, isOutput=False)
    bias_ext = nc.declare_dram_parameter("bias", [P, 522], F32, isOutput=False)
    out_ext = nc.declare_dram_parameter("outT", [DIM, TH], F32, isOutput=True)

    with tile.TileContext(nc) as tc:
        with (
            tc.tile_pool(name="persist", bufs=1) as pp,
            tc.tile_pool(name="stream", bufs=3) as sp,
            tc.tile_pool(name="evac", bufs=3) as ep,
        ):
            # ---- resident SBUF tensors (same pre-tiled layouts) ----
            xt = pp.tile([P, NT, ND, P], F32R, tag="xt")
            wq = pp.tile([P, ND, HID], F32R, tag="wq")
            wkv = pp.tile([P, ND, 2 * HID], F32R, tag="wkv")
            wp = pp.tile([P, NH, DIM], F32R, tag="wp")
            bias = pp.tile([P, 522], F32, tag="bias")
            ekvk = pp.tile([P, NT, 2 * HID], F32R, tag="ekvk")  # eK | eKV
            sq = pp.tile([P, NH, TH], F32, tag="sq")
            yt = pp.tile([P, NH, TH], F32R, tag="yt")
            bq2 = bias[:, 0:NH]
            bkv = bias[:, NH:NH + 2 * HID]
            bp8 = bias[:, NH + 2 * HID:522]

            # ---- DMAs, ordered by first use (HWDGE FIFO on sync) ----
            wkv_r = wkv_ext.rearrange("p (n h) -> p n h", n=ND)
            nc.sync.dma_start(wkv[:, 0:ND // 2, :], wkv_r[:, 0:ND // 2, :])
            BB = ND * P  # elements per xt block
            nc.sync.dma_start(xt[:, 0, :, :], xt_ext[:, 0:BB])
            nc.sync.dma_start(wkv[:, ND // 2:ND, :], wkv_r[:, ND // 2:ND, :])
            nc.sync.dma_start(bias[:, :], bias_ext[:, :])
            for i in range(1, NT):
                nc.sync.dma_start(xt[:, i, :, :], xt_ext[:, i * BB:(i + 1) * BB])
            nc.sync.dma_start(wq[:, :, :],
                              wq_ext.rearrange("p (n h) -> p n h", n=ND))
            # wbias^T batches AFTER x on the same sync FIFO: issuing them
            # on a parallel queue makes the SDMA engines round-robin them
            # against the latency-critical x stream at packet granularity
            wbts = []
            for g in range(NT // WBG):
                wbt = sp.tile([P, WBG, TH], DBF, tag="wbt", bufs=2)
                nc.sync.dma_start(
                    wbt[:, :, :],
                    wbt_ext.rearrange("p (g t) -> p g t", g=NT)[
                        :, g * WBG:(g + 1) * WBG, :],
                )
                wbts.append(wbt)
            nc.sync.dma_start(wp[:, :, :],
                              wp_ext.rearrange("p (u m) -> p u m", u=NH))

            ws = pp.tile([P, CH], DBF, tag="ws")
            nc.vector.memset(ws[:, :].bitcast(F32), 0.0)

            with tc.tile_pool(name="ps1", bufs=1, space="PSUM") as ps1:
                # PE warmup: dummy matmuls with no DMA deps keep the HAM
                # activity window busy while the first x blocks stream in,
                # so the first real matmuls run at 2.4 GHz instead of 1.2
                pw = ps1.tile([P, CH], F32, tag="warm")
                for w in range(28):
                    nc.tensor.matmul(pw[:, :], ws[:, 0:P], ws[:, :],
                                     start=True, stop=True)

                # ---- phase 1a: K|V, eK, eKV (block i arrives -> tile i) ----
                for i in range(NT):
                    pkv = ps1.tile([P, 2 * HID], F32, tag="pkv", bufs=3)
                    for n in range(ND):
                        nc.tensor.matmul(
                            pkv[:, :],
                            xt[:, i, n, :],
                            wkv[:, n, :],
                            start=(n == 0),
                            stop=(n == ND - 1),
                        )
                    kvb = sp.tile([P, 2 * HID], F32, tag="kvb", bufs=2)
                    nc.vector.tensor_add(kvb[:, :], pkv[:, :], bkv[:, :])
                    nc.scalar.activation(
                        ekvk[:, i, 0:HID], kvb[:, 0:HID], AF.Exp
                    )
                    nc.vector.tensor_mul(
                        ekvk[:, i, HID:2 * HID], ekvk[:, i, 0:HID],
                        kvb[:, HID:2 * HID],
                    )

                # ---- phase 1b: Q^T = Wq^T @ x^T[:, 0:TH] ----
                pqts = [
                    ps1.tile([P, TH], F32, tag=f"pqt{u}", name=f"pqt{u}")
                    for u in range(NH)
                ]
                for u in range(NH):
                    for n in range(ND):
                        for c in range(NC_CH):
                            nc.tensor.matmul(
                                pqts[u][:, c * CH:(c + 1) * CH],
                                wq[:, n, u * P:(u + 1) * P],
                                xt[:, 4 * c:4 * (c + 1), n, :],
                                start=(n == 0),
                                stop=(n == ND - 1),
                            )
                    # sigmoid(Q+bq) = 1/(1+exp(-Q-bq)): keeps ACT on the
                    # Exp table (a Sigmoid table swap costs ~1.5us each way)
                    sge = sp.tile([P, TH], F32, tag="sge", bufs=1,
                                  name=f"sge{u}")
                    nc.scalar.activation(
                        sge[:, :], pqts[u][:, :], AF.Exp,
                        bias=bq2[:, u:u + 1], scale=-1.0,
                    )
                    nc.vector.tensor_scalar_add(sge[:, :], sge[:, :], 1.0)
                    nc.vector.reciprocal_approx_fast(sq[:, u, :], sge[:, :])

                # pre-exp the first two ew tiles so phase 2 can start
                # immediately after Q^T (ACT is FIFO)
                ews = {}
                for st in range(2):
                    ew = sp.tile([P, TH], F32R, tag="ew", bufs=2,
                                 name=f"ew{st}")
                    nc.scalar.activation(
                        ew[:, :], wbts[st // WBG][:, st % WBG, :], AF.Exp
                    )
                    ews[st] = ew

            # ---- phase 2: den^T (acc0/1) and num^T (acc2/3) ----
            with tc.tile_pool(name="ps2", bufs=1, space="PSUM") as ps2:
                # 8 one-bank accumulator tiles: acc[a][c] for quadrant a,
                # chunk c. Finer granularity gives phase 3 an 8-slot ring.
                accs = [
                    [
                        ps2.tile([P, CH], F32, tag=f"acc{a}c{c}",
                                 name=f"acc{a}c{c}")
                        for c in range(NC_CH)
                    ]
                    for a in range(4)
                ]
                for st in range(NT):
                    if st in ews:
                        ew = ews[st]
                    else:
                        ew = sp.tile([P, TH], F32R, tag="ew", bufs=2,
                                     name=f"ew{st}")
                        nc.scalar.activation(
                            ew[:, :], wbts[st // WBG][:, st % WBG, :], AF.Exp
                        )
                    for a in range(4):
                        u = a % 2
                        base = (a // 2) * HID  # 0 -> eK(den), HID -> eKV(num)
                        lh = ekvk[:, st, base + u * P: base + (u + 1) * P]
                        for c in range(NC_CH):
                            nc.tensor.matmul(
                                accs[a][c][:, :],
                                lh,
                                ew[:, c * CH:(c + 1) * CH],
                                start=(st == 0),
                                stop=(st == NT - 1),
                            )

                # ---- epilogue: Yt^T = sQ * num^T / den^T (chunked) ----
                # recips (DVE, from den psum) run alongside ACT copying num
                # to SBUF so the multiplies hit DVE's 2x fp32 SBUF mode
                nsbs = []
                for u in range(NH):
                    nsb = sp.tile([P, TH], F32, tag="nsb", bufs=2,
                                  name=f"nsb{u}")
                    for c in range(NC_CH):
                        nc.scalar.copy(nsb[:, c * CH:(c + 1) * CH],
                                       accs[2 + u][c][:, :])
                    nsbs.append(nsb)
                first = True
                for c in range(NC_CH):
                    recs = []
                    for u in range(NH):
                        r = sp.tile([P, CH], F32, tag="rec", bufs=2,
                                    name=f"rec{u}{c}")
                        nc.vector.reciprocal_approx_fast(
                            r[:, :], accs[u][c][:, :]
                        )
                        recs.append(r)
                    if first:
                        # dummy matmuls keep HAM warm across the epilogue's
                        # PE-idle window (den c0 slots just freed)
                        for a in range(2):
                            pwd = ps2.tile([P, CH], F32, tag=f"acc{a}c0",
                                           name=f"warm2{a}")
                            nc.tensor.matmul(pwd[:, :], ws[:, 0:P], ws[:, :],
                                             start=True, stop=True)
                        first = False
                    for u in range(NH):
                        cs = slice(c * CH, (c + 1) * CH)
                        tmp = sp.tile([P, CH], F32, tag="tmp", bufs=2)
                        nc.vector.tensor_mul(tmp[:, :], nsbs[u][:, cs],
                                             recs[u][:, :])
                        nc.vector.tensor_mul(yt[:, u, cs], tmp[:, :],
                                             sq[:, u, cs])

                # ---- phase 3: out^T = Wp^T @ Yt^T + bp ----
                # m-outer: each Wp stationary tile is loaded once and used
                # for both 512-chunks; psum slots recycle the 4 acc tags
                out_r = out_ext.rearrange("(m p) t -> p m t", p=P)
                ptags = [f"acc{a}c{c}" for a in range(4) for c in range(NC_CH)]
                for mg in range(NM // OG):
                    ob = ep.tile([P, OG, TH], F32, tag="ob", bufs=3,
                                 name=f"ob{mg}")
                    for k in range(OG):
                        m = mg * OG + k
                        pos = [
                            ps2.tile([P, CH], F32,
                                     tag=ptags[(2 * m + c) % 8],
                                     name=f"po{c}{m}")
                            for c in range(NC_CH)
                        ]
                        for u in range(NH):
                            for c in range(NC_CH):
                                nc.tensor.matmul(
                                    pos[c][:, :],
                                    wp[:, u, m * P:(m + 1) * P],
                                    yt[:, u, c * CH:(c + 1) * CH],
                                    start=(u == 0),
                                    stop=(u == NH - 1),
                                )
                        for c in range(NC_CH):
                            if (m + c) % 2 == 0:
                                nc.scalar.add(ob[:, k, c * CH:(c + 1) * CH],
                                              pos[c][:, :], bp8[:, m:m + 1])
                            else:
                                nc.vector.tensor_scalar_add(
                                    ob[:, k, c * CH:(c + 1) * CH],
                                    pos[c][:, :], bp8[:, m:m + 1]
                                )
                    nc.sync.dma_start(
                        out_r[:, mg * OG:(mg + 1) * OG, :],
                        ob[:, :, :],
                    )

    nc.finalize()
    return nc


_NC = None


def _get_nc():
    global _NC
    if _NC is None:
        _NC = _build()
    return _NC


def _make_in_maps(x, Wq, bq, Wk, bk, Wv, bv, Wp, bp, wbias):
    wq = _tile_rows(np.asarray(Wq, np.float32), np.float32)
    wkv = _tile_rows(
        np.concatenate([Wk, Wv], axis=1).astype(np.float32), np.float32
    )
    wp = _tile_rows(np.asarray(Wp, np.float32), np.float32)
    bias = np.zeros((P, 522), np.float32)
    bias[:, 0:NH] = -np.asarray(bq, np.float32).reshape(NH, P).T
    bias[:, NH:NH + 2 * HID] = np.concatenate([bk, bv]).astype(np.float32)
    bias[:, NH + 2 * HID:] = np.asarray(bp, np.float32).reshape(NM, P).T
    wb = np.asarray(wbias, np.float32)[:T, :T]

    in_maps = []
    for c in range(N_CORES):
        b, half = divmod(c, 2)
        toff = half * TH
        xr = np.roll(np.asarray(x[b], np.float32).T, -toff, axis=1)
        # [P, t-block i, n, col] so one 512KB DMA unlocks one K/V tile
        xt = np.ascontiguousarray(
            xr.reshape(ND, P, NT, P).transpose(1, 2, 0, 3).reshape(P, -1)
        )
        # ew^T[s_rolled, j] = wbias[toff + j, (s_rolled + toff) % T]
        wbt = np.ascontiguousarray(
            np.roll(wb[toff:toff + TH, :], -toff, axis=1).T
        )
        wbt = _tile_rows(wbt, BF16)
        in_maps.append({
            "xt": xt, "wq": wq, "wkv": wkv, "wp": wp, "wbt": wbt,
            "bias": bias,
        })
    return in_maps


def run_on_hw(in_maps, trace=False):
    nc = _get_nc()
    return run_bass_kernel_spmd(
        nc, in_maps, core_ids=list(range(N_CORES)), trace=trace
    )


def kernel(**inputs) -> np.ndarray:
    in_maps = _make_in_maps(**inputs)
    res = run_on_hw(in_maps, trace=False)
    out = np.empty((B, T, DIM), dtype=np.float32)
    for c in range(N_CORES):
        b, half = divmod(c, 2)
        toff = half * TH
        out[b, toff:toff + TH, :] = res.results[c]["outT"].T.astype(np.float32)
    return out


# revision 25
# speedup vs baseline: 1.0645x; 1.0159x over previous
"""AFT-Full forward on 8 Trainium2 NeuronCores.

Sharding: core c -> (batch b = c//2, output-time-half h = c%2).
Each core computes out[b, h*1024:(h+1)*1024, :] with no cross-core
communication. Host-side work is only layout prep (transpose / roll /
tile / dtype cast) and the final gather.

Per-core math (T=2048, D=1024, H=256, Th=1024 = this core's t-half):
  Q^T   = Wq^T @ x_b^T[:, t-half]    [H, Th]    (fp32r matmul)
  sQ    = sigmoid(Q^T + bq)
  K|V   = x_b @ [Wk|Wv]              [T, 512]   (fp32r matmul, f32 psum)
  eK    = exp(K + bk), eKV = eK*(V + bv)        stored [s, h] in SBUF
  den^T = sum_s eK[s,h] * ew^T[s,t]             (fp32r matmul)
  num^T = same with eKV                         (fp32r matmul)
  Yt^T  = sQ * num^T / den^T
  out^T = Wp^T @ Yt^T + bp           [D, Th]    (fp32r matmul)

The t-axis of x^T and the s-axis of wbias^T are rolled by -h*1024 per
core so "this core's t-half" is always columns 0:1024 of the rolled
frame; sums over s are order-invariant so the roll is harmless.

All DRAM parameters are host-pre-tiled to [128, ...] partition-major
layout so every DMA is a plain 2D copy with large contiguous runs
(HWDGE descriptor generation on the sync sequencer is the head-latency
bottleneck otherwise).
"""

import sys

for _p in ("/opt/trn_rl_repo",):
    if _p not in sys.path:
        sys.path.insert(0, _p)

import numpy as np
import ml_dtypes

import concourse.bacc as bacc
import concourse.tile as tile
from concourse import mybir
from concourse.bass_utils import run_bass_kernel_spmd

BF16 = ml_dtypes.bfloat16

B, T, DIM, HID = 4, 2048, 1024, 256
TH = T // 2          # per-core t-half
N_CORES = 8
P = 128              # partitions
ND = DIM // P        # 8 d-tiles
NT = T // P          # 16 t(/s)-tiles
NH = HID // P        # 2 h-tiles
NM = DIM // P        # 8 output dim-tiles
CH = 512             # matmul moving free-dim chunk
NC_CH = TH // CH     # 2 chunks per t-half
WBG = 4              # wbias s-tiles per batched DMA
OG = 1               # m-tiles per staged output DMA
F32 = mybir.dt.float32
F32R = mybir.dt.float32r
DBF = mybir.dt.bfloat16
F16 = mybir.dt.float16
AF = mybir.ActivationFunctionType


def _tile_rows(a, np_dtype):
    """[G*128, N] -> [128, G*N] partition-major, contiguous."""
    g = a.shape[0] // P
    return np.ascontiguousarray(
        a.reshape(g, P, a.shape[1]).transpose(1, 0, 2).reshape(P, -1)
    ).astype(np_dtype)


def _build():
    nc = bacc.Bacc(None, target_bir_lowering=False)

    xt_ext = nc.declare_dram_parameter("xt", [P, NT * ND * P], F16,
                                       isOutput=False)
    wq_ext = nc.declare_dram_parameter("wq", [P, ND * HID], F16, isOutput=False)
    wkv_ext = nc.declare_dram_parameter("wkv", [P, ND * 2 * HID], F16,
                                        isOutput=False)
    wp_ext = nc.declare_dram_parameter("wp", [P, NH * DIM], F32R, isOutput=False)
    wbt_ext = nc.declare_dram_parameter("wbt", [P, NT * TH], DBF, isOutput=False)
    bias_ext = nc.declare_dram_parameter("bias", [P, 522], F32, isOutput=False)
    out_ext = nc.declare_dram_parameter("outT", [DIM, TH], F32, isOutput=True)

    with tile.TileContext(nc) as tc:
        with (
            tc.tile_pool(name="persist", bufs=1) as pp,
            tc.tile_pool(name="stream", bufs=3) as sp,
            tc.tile_pool(name="evac", bufs=3) as ep,
        ):
            # ---- resident SBUF tensors (same pre-tiled layouts) ----
            xt = pp.tile([P, NT, ND, P], F16, tag="xt")
            wq = pp.tile([P, ND, HID], F16, tag="wq")
            wkv = pp.tile([P, ND, 2 * HID], F16, tag="wkv")
            wp = pp.tile([P, NH, DIM], F32R, tag="wp")
            bias = pp.tile([P, 522], F32, tag="bias")
            ekvk = pp.tile([P, NT, 2 * HID], F32R, tag="ekvk")  # eK | eKV
            sq = pp.tile([P, NH, TH], F32, tag="sq")
            yt = pp.tile([P, NH, TH], F32R, tag="yt")
            bq2 = bias[:, 0:NH]
            bkv = bias[:, NH:NH + 2 * HID]
            bp8 = bias[:, NH + 2 * HID:522]

            # ---- DMAs, ordered by first use (HWDGE FIFO on sync) ----
            wkv_r = wkv_ext.rearrange("p (n h) -> p n h", n=ND)
            nc.sync.dma_start(wkv[:, 0:ND // 2, :], wkv_r[:, 0:ND // 2, :])
            BB = ND * P  # elements per xt block
            nc.sync.dma_start(xt[:, 0, :, :], xt_ext[:, 0:BB])
            nc.sync.dma_start(wkv[:, ND // 2:ND, :], wkv_r[:, ND // 2:ND, :])
            nc.sync.dma_start(bias[:, :], bias_ext[:, :])
            for i in range(1, NT):
                nc.sync.dma_start(xt[:, i, :, :], xt_ext[:, i * BB:(i + 1) * BB])
            nc.sync.dma_start(wq[:, :, :],
                              wq_ext.rearrange("p (n h) -> p n h", n=ND))
            # wbias^T batches AFTER x on the same sync FIFO: issuing them
            # on a parallel queue makes the SDMA engines round-robin them
            # against the latency-critical x stream at packet granularity
            wbts = []
            for g in range(NT // WBG):
                wbt = sp.tile([P, WBG, TH], DBF, tag="wbt", bufs=2)
                nc.sync.dma_start(
                    wbt[:, :, :],
                    wbt_ext.rearrange("p (g t) -> p g t", g=NT)[
                        :, g * WBG:(g + 1) * WBG, :],
                )
                wbts.append(wbt)
            nc.sync.dma_start(wp[:, :, :],
                              wp_ext.rearrange("p (u m) -> p u m", u=NH))

            ws = pp.tile([P, CH], DBF, tag="ws")
            nc.vector.memset(ws[:, :].bitcast(F32), 0.0)

            with tc.tile_pool(name="ps1", bufs=1, space="PSUM") as ps1:
                # PE warmup: dummy matmuls with no DMA deps keep the HAM
                # activity window busy while the first x blocks stream in,
                # so the first real matmuls run at 2.4 GHz instead of 1.2
                pw = ps1.tile([P, CH], F32, tag="warm")
                for w in range(28):
                    nc.tensor.matmul(pw[:, :], ws[:, 0:P], ws[:, :],
                                     start=True, stop=True)

                # ---- phase 1a: K|V, eK, eKV (block i arrives -> tile i) ----
                for i in range(NT):
                    pkv = ps1.tile([P, 2 * HID], F32, tag="pkv", bufs=3)
                    for n in range(ND):
                        nc.tensor.matmul(
                            pkv[:, :],
                            xt[:, i, n, :],
                            wkv[:, n, :],
                            start=(n == 0),
                            stop=(n == ND - 1),
                        )
                    kvb = sp.tile([P, 2 * HID], F32, tag="kvb", bufs=2)
                    nc.vector.tensor_add(kvb[:, :], pkv[:, :], bkv[:, :])
                    nc.scalar.activation(
                        ekvk[:, i, 0:HID], kvb[:, 0:HID], AF.Exp
                    )
                    nc.vector.tensor_mul(
                        ekvk[:, i, HID:2 * HID], ekvk[:, i, 0:HID],
                        kvb[:, HID:2 * HID],
                    )

                # ---- phase 1b: Q^T = Wq^T @ x^T[:, 0:TH] ----
                pqts = [
                    ps1.tile([P, TH], F32, tag=f"pqt{u}", name=f"pqt{u}")
                    for u in range(NH)
                ]
                for u in range(NH):
                    for n in range(ND):
                        for c in range(NC_CH):
                            nc.tensor.matmul(
                                pqts[u][:, c * CH:(c + 1) * CH],
                                wq[:, n, u * P:(u + 1) * P],
                                xt[:, 4 * c:4 * (c + 1), n, :],
                                start=(n == 0),
                                stop=(n == ND - 1),
                            )
                    # sigmoid(Q+bq) = 1/(1+exp(-Q-bq)): keeps ACT on the
                    # Exp table (a Sigmoid table swap costs ~1.5us each way)
                    sge = sp.tile([P, TH], F32, tag="sge", bufs=1,
                                  name=f"sge{u}")
                    nc.scalar.activation(
                        sge[:, :], pqts[u][:, :], AF.Exp,
                        bias=bq2[:, u:u + 1], scale=-1.0,
                    )
                    nc.vector.tensor_scalar_add(sge[:, :], sge[:, :], 1.0)
                    nc.vector.reciprocal_approx_fast(sq[:, u, :], sge[:, :])

                # pre-exp the first two ew tiles so phase 2 can start
                # immediately after Q^T (ACT is FIFO)
                ews = {}
                for st in range(2):
                    ew = sp.tile([P, TH], F32R, tag="ew", bufs=2,
                                 name=f"ew{st}")
                    nc.scalar.activation(
                        ew[:, :], wbts[st // WBG][:, st % WBG, :], AF.Exp
                    )
                    ews[st] = ew

            # ---- phase 2: den^T (acc0/1) and num^T (acc2/3) ----
            with tc.tile_pool(name="ps2", bufs=1, space="PSUM") as ps2:
                # 8 one-bank accumulator tiles: acc[a][c] for quadrant a,
                # chunk c. Finer granularity gives phase 3 an 8-slot ring.
                accs = [
                    [
                        ps2.tile([P, CH], F32, tag=f"acc{a}c{c}",
                                 name=f"acc{a}c{c}")
                        for c in range(NC_CH)
                    ]
                    for a in range(4)
                ]
                for st in range(NT):
                    if st in ews:
                        ew = ews[st]
                    else:
                        ew = sp.tile([P, TH], F32R, tag="ew", bufs=2,
                                     name=f"ew{st}")
                        nc.scalar.activation(
                            ew[:, :], wbts[st // WBG][:, st % WBG, :], AF.Exp
                        )
                    for a in range(4):
                        u = a % 2
                        base = (a // 2) * HID  # 0 -> eK(den), HID -> eKV(num)
                        lh = ekvk[:, st, base + u * P: base + (u + 1) * P]
                        for c in range(NC_CH):
                            nc.tensor.matmul(
                                accs[a][c][:, :],
                                lh,
                                ew[:, c * CH:(c + 1) * CH],
                                start=(st == 0),
                                stop=(st == NT - 1),
                            )

                # ---- epilogue: Yt^T = sQ * num^T / den^T (chunked) ----
                # recips (DVE, from den psum) run alongside ACT copying num
                # to SBUF so the multiplies hit DVE's 2x fp32 SBUF mode
                nsbs = []
                for u in range(NH):
                    nsb = sp.tile([P, TH], F32, tag="nsb", bufs=2,
                                  name=f"nsb{u}")
                    for c in range(NC_CH):
                        nc.scalar.copy(nsb[:, c * CH:(c + 1) * CH],
                                       accs[2 + u][c][:, :])
                    nsbs.append(nsb)
                first = True
                for c in range(NC_CH):
                    recs = []
                    for u in range(NH):
                        r = sp.tile([P, CH], F32, tag="rec", bufs=2,
                                    name=f"rec{u}{c}")
                        nc.vector.reciprocal_approx_fast(
                            r[:, :], accs[u][c][:, :]
                        )
                        recs.append(r)
                    if first:
                        # dummy matmuls keep HAM warm across the epilogue's
                        # PE-idle window (den c0 slots just freed)
                        for a in range(2):
                            pwd = ps2.tile([P, CH], F32, tag=f"acc{a}c0",
                                           name=f"warm2{a}")
                            nc.tensor.matmul(pwd[:, :], ws[:, 0:P], ws[:, :],
                                             start=True, stop=True)
                        first = False
                    for u in range(NH):
                        cs = slice(c * CH, (c + 1) * CH)
                        tmp = sp.tile([P, CH], F32, tag="tmp", bufs=2)
                        nc.vector.tensor_mul(tmp[:, :], nsbs[u][:, cs],
                                             recs[u][:, :])
                        nc.vector.tensor_mul(yt[:, u, cs], tmp[:, :],
                                             sq[:, u, cs])

                # ---- phase 3: out^T = Wp^T @ Yt^T + bp ----
                # m-outer: each Wp stationary tile is loaded once and used
                # for both 512-chunks; psum slots recycle the 4 acc tags
                out_r = out_ext.rearrange("(m p) t -> p m t", p=P)
                ptags = [f"acc{a}c{c}" for a in range(4) for c in range(NC_CH)]
                for mg in range(NM // OG):
                    ob = ep.tile([P, OG, TH], F32, tag="ob", bufs=3,
                                 name=f"ob{mg}")
                    for k in range(OG):
                        m = mg * OG + k
                        pos = [
                            ps2.tile([P, CH], F32,
                                     tag=ptags[(2 * m + c) % 8],
                                     name=f"po{c}{m}")
                            for c in range(NC_CH)
                        ]
                        for u in range(NH):
                            for c in range(NC_CH):
                                nc.tensor.matmul(
                                    pos[c][:, :],
                                    wp[:, u, m * P:(m + 1) * P],
                                    yt[:, u, c * CH:(c + 1) * CH],
                                    start=(u == 0),
                                    stop=(u == NH - 1),
                                )
                        for c in range(NC_CH):
                            if (m + c) % 2 == 0:
                                nc.scalar.add(ob[:, k, c * CH:(c + 1) * CH],
                                              pos[c][:, :], bp8[:, m:m + 1])
                            else:
                                nc.vector.tensor_scalar_add(
                                    ob[:, k, c * CH:(c + 1) * CH],
                                    pos[c][:, :], bp8[:, m:m + 1]
                                )
                    nc.sync.dma_start(
                        out_r[:, mg * OG:(mg + 1) * OG, :],
                        ob[:, :, :],
                    )

    nc.finalize()
    return nc


_NC = None


def _get_nc():
    global _NC
    if _NC is None:
        _NC = _build()
    return _NC


def _make_in_maps(x, Wq, bq, Wk, bk, Wv, bv, Wp, bp, wbias):
    wq = _tile_rows(np.asarray(Wq, np.float32), np.float16)
    wkv = _tile_rows(
        np.concatenate([Wk, Wv], axis=1).astype(np.float32), np.float16
    )
    wp = _tile_rows(np.asarray(Wp, np.float32), np.float32)
    bias = np.zeros((P, 522), np.float32)
    bias[:, 0:NH] = -np.asarray(bq, np.float32).reshape(NH, P).T
    bias[:, NH:NH + 2 * HID] = np.concatenate([bk, bv]).astype(np.float32)
    bias[:, NH + 2 * HID:] = np.asarray(bp, np.float32).reshape(NM, P).T
    wb = np.asarray(wbias, np.float32)[:T, :T]

    in_maps = []
    for c in range(N_CORES):
        b, half = divmod(c, 2)
        toff = half * TH
        xr = np.roll(np.asarray(x[b], np.float32).T, -toff, axis=1)
        # [P, t-block i, n, col] so one 512KB DMA unlocks one K/V tile
        xt = np.ascontiguousarray(
            xr.reshape(ND, P, NT, P).transpose(1, 2, 0, 3).reshape(P, -1)
        ).astype(np.float16)
        # ew^T[s_rolled, j] = wbias[toff + j, (s_rolled + toff) % T]
        wbt = np.ascontiguousarray(
            np.roll(wb[toff:toff + TH, :], -toff, axis=1).T
        )
        wbt = _tile_rows(wbt, BF16)
        in_maps.append({
            "xt": xt, "wq": wq, "wkv": wkv, "wp": wp, "wbt": wbt,
            "bias": bias,
        })
    return in_maps


def run_on_hw(in_maps, trace=False):
    nc = _get_nc()
    return run_bass_kernel_spmd(
        nc, in_maps, core_ids=list(range(N_CORES)), trace=trace
    )


def kernel(**inputs) -> np.ndarray:
    in_maps = _make_in_maps(**inputs)
    res = run_on_hw(in_maps, trace=False)
    out = np.empty((B, T, DIM), dtype=np.float32)
    for c in range(N_CORES):
        b, half = divmod(c, 2)
        toff = half * TH
        out[b, toff:toff + TH, :] = res.results[c]["outT"].T.astype(np.float32)
    return out
